# revision 15
# baseline (speedup 1.0000x reference)
"""LocalFrameAttentionWithDiffuser on 8 TRN2 NeuronCores.

Sharding: head-parallel. Each core computes 2 of the 16 heads end-to-end
(QKV projection for its 128 hd-dims, chunked local attention, partial
output projection Y_c = O_c @ Wo[c-slice]); the host sums the 8 partial
Y tensors and adds the bias once.

Shapes (hardcoded from the problem):
  x [1,16,256,1024] -> tokens T=4096, D=1024, H=16 heads, HD=64,
  chunks C=4 of L=1024 tokens; chunk i attends to chunks {i-1, i}
  (chunk 0 only to itself).

v2 design notes (cost-model driven):
  - bf16 datapath everywhere on SBUF (PSUM accumulates fp32): matmul
    rate is unchanged vs float32r at moving>=256, but bf16 keeps the
    full 1 cyc/row rate at small moving sizes, halves DMA bytes and
    speeds DVE copies.
  - scores S^T [ctx, q] per (chunk, head): PSUM tile [128, 1024]
    holds TWO 128-ctx tiles for 512 queries; ONE exp activation
    covers both (fewer Act instructions - Act is the bottleneck engine
    at ~116us; every score element must pass through it).
  - AV is emitted "flipped": stationary = A tile [128 ctx, 128 q],
    moving = V' [128 ctx, 65] (64 hd dims + ones column -> softmax
    denominator lands in column 64). Moving size 65 instead of 512
    halves the PE cost of AV.
  - normalization is a per-partition reciprocal + tensor_scalar mul
    into [tok, hd] staging, then a PE transpose back to [hd, tok] for
    the output projection.
  - V is projected directly in [tok, hd] orientation (stationary =
    x^T tile, moving = Wv tile) - no V transposes.
  - single interleaved emission schedule: projection chains /
    transposes / output-projection tiles are spread between attention
    pair-events as PE filler so the PE never idles (the cost model
    halves PE speed for 3us after any idle gap).
  - y partial [T, D] fp32 is DMA'd straight from PSUM (no engine copy);
    x/weights ship as bf16; DMAs are coalesced (the HWDGE device is
    serial at ~630ns per dma_start).
"""

from contextlib import ExitStack

import numpy as np

import concourse.bass as bass
import concourse.tile as tile
from concourse import bacc, mybir
from concourse.bass_utils import run_bass_kernel_spmd

F32 = mybir.dt.float32
BF16 = mybir.dt.bfloat16

B, F, N, D = 1, 16, 256, 1024
H, HD = 16, 64
CS = 4
C = F // CS            # 4 chunks
L = CS * N             # 1024 tokens per chunk
T = F * N              # 4096 tokens
NCORES = 8
HPC = H // NCORES      # 2 heads per core
HDB = HPC * HD         # 128 hd dims per core
SCALE = 1.0 / np.sqrt(HD)

NDT = D // 128         # 8 contraction tiles for projections
NQT = T // 512         # 8 query-projection tiles (512 tokens each)
NCT = T // 128         # 32 ctx tiles of 128 tokens
NTB = T // 128         # 32 token tiles (output side)

# ctx tiles seen by chunk c (128-token tiles, global index)
CTS = [list(range(max(0, 8 * (c - 1)), 8 * (c + 1))) for c in range(C)]
PAIRS = [[(t[i], t[i + 1]) for i in range(0, len(t), 2)] for t in CTS]
# attention pair events: (c, th, h, p); th = 512-query half of the chunk
EVENTS = [
    (c, th, h, p)
    for c in range(C)
    for th in (0, 1)
    for h in range(HPC)
    for p in range(len(PAIRS[c]))
]
assert len(EVENTS) == 112

# static PE filler schedule: event index -> list of (kind, idx)
# kinds: Q/K = 512-token q/k projection chain j, V = 128-token v
# projection ct, P = post work for token tile tb (transpose + out-proj)
FILLER = {
    0: [("V", 0), ("V", 1), ("V", 2)],
    1: [("K", 1), ("V", 3)],
    2: [("V", 4), ("V", 5)],
    3: [("V", 6), ("V", 7)],
    4: [("Q", 1)],
    5: [("K", 2)],
    6: [("Q", 2)],
    7: [("K", 3)],
    8: [("Q", 3)],
    9: [("V", 8)], 10: [("V", 9)], 11: [("V", 10)], 12: [("V", 11)],
    13: [("P", 0)], 14: [("P", 1)], 15: [("P", 2)],
    16: [("V", 12)], 17: [("V", 13)], 18: [("V", 14)], 19: [("V", 15)],
    20: [("P", 3)], 21: [("P", 4)], 22: [("P", 5)], 23: [("P", 6)],
    24: [("P", 7)],
    25: [("Q", 4)], 26: [("K", 4)], 27: [("Q", 5)], 28: [("K", 5)],
    33: [("P", 8)], 34: [("P", 9)], 35: [("P", 10)], 36: [("P", 11)],
    37: [("V", 16)], 38: [("V", 17)], 39: [("V", 18)], 40: [("V", 19)],
    48: [("V", 20)], 49: [("V", 21)], 50: [("V", 22)], 51: [("V", 23)],
    52: [("P", 12)], 53: [("P", 13)], 54: [("P", 14)], 55: [("P", 15)],
    56: [("Q", 6)], 57: [("K", 6)], 58: [("Q", 7)], 59: [("K", 7)],
    65: [("P", 16)], 66: [("P", 17)], 67: [("P", 18)], 68: [("P", 19)],
    69: [("V", 24)], 70: [("V", 25)], 71: [("V", 26)], 72: [("V", 27)],
    80: [("V", 28)], 81: [("V", 29)], 82: [("V", 30)], 83: [("V", 31)],
    84: [("P", 20)], 85: [("P", 21)], 86: [("P", 22)], 87: [("P", 23)],
    97: [("P", 24)], 98: [("P", 25)], 99: [("P", 26)], 100: [("P", 27)],
}


def build_kernel(nc, tc, outs, ins, ctx):
    xt, wq, wk, wv, wo, ident = (
        ins["xt"], ins["wq"], ins["wk"], ins["wv"], ins["wo"], ins["ident"],
    )
    y = outs["y"]

    # ---- SBUF pools ----
    wpool = ctx.enter_context(tc.tile_pool(name="weights", bufs=1))
    xpool = ctx.enter_context(tc.tile_pool(name="xtiles", bufs=1))
    qk_pool = ctx.enter_context(tc.tile_pool(name="qk", bufs=1))
    v_pool = ctx.enter_context(tc.tile_pool(name="vsb", bufs=1))
    a_pool = ctx.enter_context(tc.tile_pool(name="attn", bufs=4))
    osb_pool = ctx.enter_context(tc.tile_pool(name="osb", bufs=12))
    r_pool = ctx.enter_context(tc.tile_pool(name="recip", bufs=4))
    ot_pool = ctx.enter_context(tc.tile_pool(name="ot", bufs=1))
    ysb_pool = ctx.enter_context(tc.tile_pool(name="ysb", bufs=2))
    # PSUM: s/y 2x2 banks + o 2x1 + p 2x1 = 8 banks exactly
    pp = ctx.enter_context(tc.tile_pool(name="pp", bufs=1, space="PSUM"))

    # persistent SBUF tiles
    wq_sb = wpool.tile([128, D], BF16, name="wqsb")
    wk_sb = wpool.tile([128, D], BF16, name="wksb")
    wv_sb = wpool.tile([128, D], BF16, name="wvsb")
    wo_sb = wpool.tile([128, D], BF16, name="wosb")
    id_sb = wpool.tile([128, 128], BF16, name="idsb")
    xt_t = [xpool.tile([128, T], BF16, name=f"xt{d}") for d in range(NDT)]
    qt_sb = qk_pool.tile([128, T], BF16, name="qtsb")   # [2 heads x 64, T]
    kt_sb = qk_pool.tile([128, T], BF16, name="ktsb")
    # V per head: [128 ctx-token partitions, 32 ctx tiles, 64 hd + ones col]
    v_sb = [v_pool.tile([128, NCT, HD + 1], BF16, name=f"vsb{h}")
            for h in range(HPC)]
    ot_sb = ot_pool.tile([128, T], BF16, name="otsb")   # O^T normalized

    # ---- input DMAs (HWDGE is serial: few, large transfers; ordered so
    # the first projection chains unblock as early as possible) ----
    nc.sync.dma_start(wq_sb[:], wq[:, :])
    nc.sync.dma_start(wk_sb[:], wk[:, :])
    for d in range(NDT):   # tokens 0..511 of every contraction tile
        nc.sync.dma_start(xt_t[d][:, 0:512], xt[d * 128:(d + 1) * 128, 0:512])
    nc.sync.dma_start(wv_sb[:], wv[:, :])
    nc.sync.dma_start(wo_sb[:], wo[:, :])
    nc.sync.dma_start(id_sb[:], ident[:, :])
    for d in range(NDT):   # tokens 512..1023
        nc.sync.dma_start(xt_t[d][:, 512:1024], xt[d * 128:(d + 1) * 128, 512:1024])
    for d in range(NDT):   # rest
        nc.sync.dma_start(xt_t[d][:, 1024:T], xt[d * 128:(d + 1) * 128, 1024:T])

    # ones columns of V (softmax denominator rides the AV matmul)
    for h in range(HPC):
        nc.vector.memset(v_sb[h][:, :, HD], 1.0)

    # ---- emission helpers ----
    def emit_Q(j, which):
        w_sb, dst = (wq_sb, qt_sb) if which == "Q" else (wk_sb, kt_sb)
        ps = pp.tile([128, 512], F32, tag="p", bufs=2, name=f"{which}ps{j}")
        for d in range(NDT):
            nc.tensor.matmul(
                ps[:], w_sb[:, d * 128:(d + 1) * 128],
                xt_t[d][:, j * 512:(j + 1) * 512],
                start=(d == 0), stop=(d == NDT - 1),
            )
        nc.vector.tensor_copy(dst[:, j * 512:(j + 1) * 512], ps[:])

    def emit_V(ct):
        ps = pp.tile([128, 128], F32, tag="p", bufs=2, name=f"vps{ct}")
        for d in range(NDT):
            nc.tensor.matmul(
                ps[:], xt_t[d][:, ct * 128:(ct + 1) * 128],
                wv_sb[:, d * 128:(d + 1) * 128],
                start=(d == 0), stop=(d == NDT - 1),
            )
        for h in range(HPC):
            nc.vector.tensor_copy(
                v_sb[h][:, ct, 0:HD], ps[:, h * HD:(h + 1) * HD])

    osb_tiles = {}

    def get_osb(tb):
        if tb not in osb_tiles:
            osb_tiles[tb] = osb_pool.tile(
                [128, 128], BF16, tag="osb", name=f"osb{tb}")
        return osb_tiles[tb]

    ysb_tiles = {}

    def emit_post(tb):
        g = tb // 4
        if g not in ysb_tiles:
            ysb_tiles[g] = ysb_pool.tile(
                [128, 4, D], BF16, tag="ysb", name=f"ysb{g}")
        ysb = ysb_tiles[g]
        t_ps = pp.tile([128, 128], BF16, tag="p", bufs=2, name=f"tp{tb}")
        nc.tensor.transpose(t_ps[:], osb_tiles[tb][:], id_sb[:])
        nc.vector.tensor_copy(ot_sb[:, tb * 128:(tb + 1) * 128], t_ps[:])
        for dh in range(D // 512):
            # yp shares the "s" pool slots: the slot-rotation WAR dep forces
            # the scheduler to run the post pipeline interleaved with the
            # attention stream instead of deferring it to a serialized tail
            yp = pp.tile([128, 512], F32, tag="s", bufs=2, name=f"yp{tb}_{dh}")
            nc.tensor.matmul(
                yp[:],
                ot_sb[:, tb * 128:(tb + 1) * 128],
                wo_sb[:, dh * 512:(dh + 1) * 512],
                start=True, stop=True,
            )
            nc.vector.tensor_copy(
                ysb[:, tb % 4, dh * 512:(dh + 1) * 512], yp[:])
        if tb % 4 == 3:
            # y is [8, 4, 128, 1024]; dst iterates [p][tb][d] to match the
            # SBUF staging tile's element order
            nc.sync.dma_start(
                y[g:g + 1].transpose([0, 2, 1, 3]), ysb[:])

    def emit_filler(kind, idx):
        if kind in ("Q", "K"):
            emit_Q(idx, kind)
        elif kind == "V":
            emit_V(idx)
        else:
            emit_post(idx)

    def emit_spair(e, c, th, h, p):
        ct0, ct1 = PAIRS[c][p]
        tok0 = c * L + th * 512
        hr = slice(h * HD, (h + 1) * HD)
        s_t = pp.tile([128, 1024], F32, tag="s", bufs=2, name=f"s{e}")
        nc.tensor.matmul(
            s_t[:, 0:512], kt_sb[hr, ct0 * 128:(ct0 + 1) * 128],
            qt_sb[hr, tok0:tok0 + 512], start=True, stop=True)
        nc.tensor.matmul(
            s_t[:, 512:1024], kt_sb[hr, ct1 * 128:(ct1 + 1) * 128],
            qt_sb[hr, tok0:tok0 + 512], start=True, stop=True)
        a_t = a_pool.tile([128, 1024], BF16, tag="a", name=f"a{e}")
        nc.scalar.activation(
            a_t[:], s_t[:], mybir.ActivationFunctionType.Exp, scale=SCALE)
        return a_t

    o_tiles = {}

    def emit_oinit(c, th, h):
        # matmul start=True zeroes the WHOLE psum bank, so the four
        # interleaved qb accumulation regions cannot each use start.
        # Zero the tile once on DVE and accumulate with start=False.
        o_t = pp.tile(
            [128, 4, HD + 1], F32, tag="o", bufs=2, name=f"o{c}_{th}_{h}")
        o_tiles[(c, th, h)] = o_t
        nc.vector.memset(o_t[:], 0.0)

    def emit_av(c, th, h, p, a_t):
        npair = len(PAIRS[c])
        o_t = o_tiles[(c, th, h)]
        for ci, ct in enumerate(PAIRS[c][p]):
            for qb in range(4):
                nc.tensor.matmul(
                    o_t[:, qb, :],
                    a_t[:, ci * 512 + qb * 128: ci * 512 + (qb + 1) * 128],
                    v_sb[h][:, ct, :],
                    start=False,
                    stop=(p == npair - 1 and ci == 1),
                    skip_group_check=True,
                )

    def emit_normalize(c, th, h):
        o_t = o_tiles[(c, th, h)]
        r = r_pool.tile([128, 4], F32, tag="r", name=f"r{c}_{th}_{h}")
        nc.vector.reciprocal(r[:], o_t[:, :, HD])
        for qb in range(4):
            tb = c * 8 + th * 4 + qb
            osb = get_osb(tb)
            nc.vector.tensor_scalar_mul(
                osb[:, h * HD:(h + 1) * HD], o_t[:, qb, 0:HD], r[:, qb:qb + 1])

    # ---- main interleaved emission ----
    emit_Q(0, "Q")
    emit_Q(0, "K")

    pending_av = None   # (c, th, h, p, a_t) awaiting emission (lag 1)
    prev_cthh = None
    for e, (c, th, h, p) in enumerate(EVENTS):
        if prev_cthh is not None and prev_cthh != (c, th, h):
            # drain previous (c, th, h): tail AV + normalization
            pc, pth, ph, pp_, pa = pending_av
            emit_av(pc, pth, ph, pp_, pa)
            pending_av = None
            emit_normalize(pc, pth, ph)
        if p == 0:
            emit_oinit(c, th, h)
        a_t = emit_spair(e, c, th, h, p)
        if pending_av is not None:
            emit_av(*pending_av)
        pending_av = (c, th, h, p, a_t)
        prev_cthh = (c, th, h)
        for kind, idx in FILLER.get(e, ()):
            emit_filler(kind, idx)

    # tail
    pc, pth, ph, pp_, pa = pending_av
    emit_av(pc, pth, ph, pp_, pa)
    emit_normalize(pc, pth, ph)
    for tb in range(28, 32):
        emit_post(tb)


_CACHE = {}


def _build():
    if "nc" in _CACHE:
        return _CACHE["nc"]
    nc = bacc.Bacc(
        "TRN2",
        target_bir_lowering=False,
        debug=False,
        enable_asserts=False,
        num_devices=NCORES,
    )
    ins = {
        "xt": nc.dram_tensor("xt", [D, T], BF16, kind="ExternalInput").ap(),
        "wq": nc.dram_tensor("wq", [128, D], BF16, kind="ExternalInput").ap(),
        "wk": nc.dram_tensor("wk", [128, D], BF16, kind="ExternalInput").ap(),
        "wv": nc.dram_tensor("wv", [128, D], BF16, kind="ExternalInput").ap(),
        "wo": nc.dram_tensor("wo", [128, D], BF16, kind="ExternalInput").ap(),
        "ident": nc.dram_tensor("ident", [128, 128], BF16, kind="ExternalInput").ap(),
    }
    outs = {"y": nc.dram_tensor(
        "y", [T // 512, 4, 128, D], BF16, kind="ExternalOutput").ap()}
    with tile.TileContext(nc, trace_sim=False) as tc:
        with ExitStack() as kctx:
            build_kernel(nc, tc, outs, ins, kctx)
    nc.compile()
    _CACHE["nc"] = nc
    return nc


def make_in_maps(x, Wq, Wk, Wv, Wo, bo):
    BF = mybir.dt.np(mybir.dt.bfloat16)
    xt = np.ascontiguousarray(
        np.asarray(x, dtype=np.float32).reshape(T, D).T
    ).astype(BF)
    ident = np.eye(128, dtype=np.float32).astype(BF)

    def pack(w):  # [1024, 128] -> [128, 8*128], ktile d at cols d*128
        return np.ascontiguousarray(np.concatenate(
            [w[d * 128:(d + 1) * 128, :] for d in range(NDT)], axis=1
        )).astype(BF)

    in_maps = []
    for core in range(NCORES):
        hs = slice(core * HDB, (core + 1) * HDB)
        in_maps.append({
            "xt": xt,
            "wq": pack(np.asarray(Wq, np.float32)[:, hs]),
            "wk": pack(np.asarray(Wk, np.float32)[:, hs]),
            "wv": pack(np.asarray(Wv, np.float32)[:, hs]),
            "wo": np.ascontiguousarray(
                np.asarray(Wo, np.float32)[hs, :]).astype(BF),
            "ident": ident,
        })
    return in_maps


def kernel(x, Wq, Wk, Wv, Wo, bo, _trace=False, _tmpdir=None):
    nc = _build()
    in_maps = make_in_maps(x, Wq, Wk, Wv, Wo, bo)
    res = run_bass_kernel_spmd(
        nc, in_maps, core_ids=list(range(NCORES)),
        trace=_trace, tmpdir=_tmpdir,
        **({"trace_cores": list(range(NCORES))} if _trace else {}),
    )
    if _trace:
        kernel.last_results = res
    y = np.zeros((T, D), dtype=np.float32)
    for r in res.results:
        y += np.asarray(r["y"], dtype=np.float32).reshape(T, D)
    y += np.asarray(bo, dtype=np.float32).reshape(1, D)
    return y.reshape(B, F, N, D)
